# revision 3
# baseline (speedup 1.0000x reference)
"""Trainium2 Bass kernel v3 for nn_AttentionLayer_88399016887055.

Math (per head h, B=1):
  w = W[h] @ ft + b[h]                               # [N]
  s_ij = leaky_relu(w_i + w_j, 0.2) + bias[i, j]
  a = softmax_rows(s)                                # [N, N]
  t[h] = a @ a^T                                     # [N, N]
  out[:, i, h*N + k] = t[h][i, k]

v3 design (all fp16 data path; cost-model-guided):
  - lrelu(z) = 0.6 z + 0.4 |z| computed on the DVE via abs_max; the Act
    engine runs ONLY Exp (plus Copy, same act-table set -> no table swaps).
  - C[j, i] = exp(0.6 w_i + 0.4|w_i + w_j| + bias[j, i] + 0.6 w_j - SHIFT)
    stored fp16 (matmuls at 1 cyc/row; fp8 was numerically insufficient).
  - column sums via matmuls with C as STATIONARY and a ones column moving
    (ap_size=1 -> ~free on the PE), accumulated in one PSUM tile across all
    strips, giving r in rcols layout directly.
  - two phases of 16 j-strips (fp16 C half stays SBUF-resident); phase-A
    G partials spill to DRAM in fp16; phase-B elementwise overlaps phase-A
    G matmuls.
  - outputs only the DIRECT diagonal blocks d in [0,16] per ib in a compact
    [2048, 2176] fp16 layout; the host reconstructs mirror blocks by
    transposition (t is symmetric) and upcasts.
  - sharding: 8 cores = (head h, row-half off); off cores get ft/bias rolled
    by -2048 so one SPMD program serves all cores.
"""

import sys

import numpy as np

sys.path.insert(0, "/opt/trn_rl_repo")

import concourse.bass as bass
import concourse.bacc as bacc
import concourse.mybir as mybir
import concourse.tile as tile
from concourse.bass_utils import run_bass_kernel_spmd
from concourse.tile_rust import add_dep_helper

F32 = mybir.dt.float32
F16 = mybir.dt.float16
AF = mybir.ActivationFunctionType
ALU = mybir.AluOpType

N = 4096
FT = 512
H = 4
NCORES = 8
SHIFT = 6.0

NB = N // 128          # 32 j/i strips
NH = NB // 2           # 16 C tiles (two ping-pong sets of NB/NSUB)
HALF = N // 2          # 2048 output rows per core
DMAX = NB // 2         # 16: diagonal blocks d in [0, DMAX]
GCOLS = (DMAX + 1) * 128   # 2176 compact output columns
NSUB = 4               # G-accumulation subphases (C sets ping-pong)


def build_kernel_v3(n=N, ft_dim=FT):
    nf = ft_dim // 128

    nc = bacc.Bacc(None, target_bir_lowering=False, debug=False)
    ftr = nc.dram_tensor("ftr", [ft_dim, n], F16, kind="ExternalInput")
    biasT = nc.dram_tensor("biasT", [n, n], F16, kind="ExternalInput")
    wh = nc.dram_tensor("wh", [1, ft_dim], F16, kind="ExternalInput")
    bh = nc.dram_tensor("bh", [1, 1], F32, kind="ExternalInput")
    out = nc.dram_tensor("out", [HALF, GCOLS], F16, kind="ExternalOutput")
    w_scr = nc.dram_tensor("w_scr", [1, n], F16)
    r_scr = nc.dram_tensor("r_scr", [1, n], F16)
    g_scr = nc.dram_tensor("g_scr", [HALF, GCOLS], F16)

    with tile.TileContext(nc) as tc:
        with tc.tile_pool(name="persist", bufs=1) as P, \
                tc.tile_pool(name="work", bufs=1) as WK, \
                tc.tile_pool(name="pwork", bufs=1, space="PSUM") as PW:
            ones_c = P.tile([128, 1], F16, tag="ones_c")
            whT = P.tile([128, nf], F16, tag="whT")
            bhs = P.tile([128, 1], F32, tag="bhs")
            wcols = P.tile([128, NB], F32, tag="wcols")
            wc8 = P.tile([128, NB], F32, tag="wc8")
            expb = P.tile([128, NB], F32, tag="expb")
            rcols = P.tile([128, NB], F32, tag="rcols")
            scol = P.tile([128, NB], F32, tag="scol")
            w816 = P.tile([128, NB], F16, tag="w816")
            r16 = P.tile([128, NB], F16, tag="r16")
            w8row = P.tile([128, n], F16, tag="w8row")
            rbc = P.tile([128, n], F16, tag="rbc")
            C = [P.tile([128, n], F16, tag=f"C{s}", name=f"C{s}")
                 for s in range(NH)]

            nc.vector.memset(ones_c[:], 1.0)
            nc.sync.dma_start(whT[:], wh[0, :].rearrange("(f p) -> p f", p=128))
            nc.sync.dma_start(bhs[:], bh[0, :][None, :].to_broadcast((128, 1)))

            # ---- w in wcols layout: w[p, t] for j = t*128 + p ----
            # lhsT = ft[f-strip, j-block] (stationary), rhs = whT column.
            # Per-column accumulation groups in one PSUM bank are unsafe on
            # HW (a matmul `start` zeroes the whole 2 KB bank row), so the
            # accumulator is DVE-zeroed once and all matmuls accumulate.
            pw_w = PW.tile([128, NB], F32, tag="psum_s", name="pw_w")
            nc.vector.memset(pw_w[:], 0.0)
            for f in range(nf):
                ftile = WK.tile([128, n], F16, tag="t", bufs=2,
                                name=f"ft{f}")
                nc.sync.dma_start(ftile[:], ftr[f * 128:(f + 1) * 128, :])
                for t in range(NB):
                    nc.tensor.matmul(
                        pw_w[:, t:t + 1],
                        ftile[:, t * 128:(t + 1) * 128],
                        whT[:, f:f + 1],
                        start=False, stop=(f == nf - 1),
                        skip_group_check=True)
            nc.scalar.activation(wcols[:], pw_w[:], AF.Identity, bias=bhs[:, 0:1])
            nc.vector.tensor_scalar(expb[:], wcols[:], 0.2, -SHIFT,
                                    op0=ALU.mult, op1=ALU.add)
            nc.vector.tensor_scalar(wc8[:], wcols[:], 0.8, None, op0=ALU.mult)
            nc.vector.tensor_copy(w816[:], wc8[:])
            st_w = nc.sync.dma_start(
                w_scr[0, :].rearrange("(t p) -> p t", p=128), w816[:])
            ld_wr = nc.sync.dma_start(
                w8row[:], w_scr[0, :][None, :].to_broadcast((128, n)))
            add_dep_helper(ld_wr.ins, st_w.ins, reason="w_scr RAW")

            # persistent PSUM: column-sum accumulator (lives both phases);
            # shares the ring slot with pw_w (disjoint lifetimes, in order).
            # DVE-zeroed once; all sum matmuls accumulate without `start`.
            psum_s = PW.tile([128, NB], F32, tag="psum_s", name="psum_s")
            nc.vector.memset(psum_s[:], 0.0)

            g_store = {}
            K = NB // NSUB          # strips per subphase
            assert 2 * K <= NH, "C ping-pong sets must fit the 16-tile pool"

            def emit_elem(sub, s):
                """Elementwise for strip js = sub*K+s (DVE x2 + Act exp)."""
                js = sub * K + s
                cs = C[(sub % 2) * K + s]
                bt = WK.tile([128, n], F16, tag="bt", bufs=2,
                             name=f"bt{sub}_{s}")
                nc.sync.dma_start(
                    bt[:], biasT[js * 128:(js + 1) * 128, :])
                t_ = WK.tile([128, n], F16, tag="t", bufs=2,
                             name=f"t{sub}_{s}")
                # 0.8 relu(w_i + w_j) via (add, max) on pre-scaled rows
                # (lrelu(z) = 0.2 z + 0.8 relu(z); 0.2 w_i is host-folded
                # into bias, 0.2 w_j rides the exp bias)
                nc.vector.tensor_scalar(
                    t_[:], w8row[:], wc8[:, js:js + 1], 0.0,
                    op0=ALU.add, op1=ALU.max)
                nc.vector.tensor_add(t_[:], t_[:], bt[:])
                nc.scalar.activation(
                    cs[:], t_[:], AF.Exp, bias=expb[:, js:js + 1])

            def emit_sums(sub, s):
                """Column sums of strip: C stationary, ones moving (ap=1).
                Deferred to after the strip's exp is long done so these PE
                ops never head-of-line block the in-order PE queue."""
                js = sub * K + s
                cs = C[(sub % 2) * K + s]
                for ibs in range(NB):
                    nc.tensor.matmul(
                        psum_s[:, ibs:ibs + 1],
                        cs[:, ibs * 128:(ibs + 1) * 128],
                        ones_c[:],
                        start=False, stop=(js == NB - 1),
                        skip_group_check=True)

            # prologue: subphase-0 elementwise (no G running yet)
            for s in range(K):
                emit_elem(0, s)
                emit_sums(0, s)

            for sub in range(NSUB):
                cset = (sub % 2) * K
                if sub > 0:
                    # sums for this sub's strips (elem ran last window)
                    for s in range(K):
                        emit_sums(sub, s)

                if sub == NSUB - 1:
                    # r = 1/s in rcols layout; broadcast row via DRAM trip
                    nc.scalar.copy(scol[:], psum_s[:])
                    nc.vector.reciprocal(rcols[:], scol[:])
                    nc.vector.tensor_copy(r16[:], rcols[:])
                    st_r = nc.sync.dma_start(
                        r_scr[0, :].rearrange("(t p) -> p t", p=128), r16[:])
                    ld_rb = nc.sync.dma_start(
                        rbc[:], r_scr[0, :][None, :].to_broadcast((128, n)))
                    add_dep_helper(ld_rb.ins, st_r.ins, reason="r_scr RAW")

                # ---- G: diagonal groups d in [0,16], compact output ----
                # next subphase's elementwise is interleaved between ib
                # groups so its DVE/Act ops slot into this window's gaps
                # group widths: 4x512 diagonal groups + the 128-wide dmax
                GW = [512, 512, 512, 512, 128]
                for ib in range(NH):
                    if sub + 1 < NSUB and ib % 2 == 1:
                        emit_elem(sub + 1, ib // 2)
                    ga = None
                    if sub > 0:
                        ga = WK.tile([128, GCOLS], F16, tag="ga", bufs=2,
                                     name=f"ga{sub}_{ib}")
                        ld = nc.sync.dma_start(
                            ga[:], g_scr[ib * 128:(ib + 1) * 128, :])
                        add_dep_helper(ld.ins, g_store[ib].ins, reason="g RAW")
                    evt = WK.tile([128, GCOLS], F16, tag="ev", bufs=3,
                                  name=f"ev{sub}_{ib}")
                    # dg-major emission: each PSUM bank's accumulation closes
                    # after K matmuls, so its evict overlaps the next bank's
                    # matmuls instead of serializing at the end of the ib set
                    for dg in range(5):
                        gw = GW[dg]
                        c0 = ib * 128 + dg * 512
                        sl = slice(dg * 512, dg * 512 + gw)
                        pt = PW.tile([128, 512], F32, tag="ps", bufs=7,
                                     name=f"ps{sub}_{ib}_{dg}")
                        for s in range(K):
                            nc.tensor.matmul(
                                pt[:, 0:gw],
                                C[cset + s][:, ib * 128:(ib + 1) * 128],
                                C[cset + s][:, c0:c0 + gw],
                                start=(s == 0), stop=(s == K - 1))
                        if sub == 0:
                            # first partial: evict on Act, spill fp16
                            nc.scalar.copy(evt[:, sl], pt[:, 0:gw])
                        else:
                            # (psum + g_prev) -> fp16 on DVE (PSUM direct)
                            nc.vector.tensor_add(evt[:, sl], pt[:, 0:gw],
                                                 ga[:, sl])
                    if sub < NSUB - 1:
                        # spill partial (store waits on evt, which waits on
                        # the ga load -> DRAM WAR is safe)
                        g_store[ib] = nc.sync.dma_start(
                            g_scr[ib * 128:(ib + 1) * 128, :], evt[:])
                    else:
                        # final: normalize and write out
                        nc.scalar.mul(evt[:], evt[:], rcols[:, ib:ib + 1])
                        c0 = ib * 128
                        nc.vector.tensor_mul(evt[:], evt[:],
                                             rbc[:, c0:c0 + GCOLS])
                        nc.sync.dma_start(
                            out[ib * 128:(ib + 1) * 128, :], evt[:])
    return nc


def make_core_inputs_v3(ft_mat, bias_mat, W, b, n=N):
    """Host-side shard prep: fp16 casts, transpose, roll, and folding of the
    0.6*w_i row term into the bias tensor (input prep; the device recomputes
    w itself for all per-partition terms)."""
    ft0 = np.asarray(ft_mat, np.float32)[0]
    bias0 = np.asarray(bias_mat, np.float32)[0]
    W32 = np.asarray(W, np.float32)
    b32 = np.asarray(b, np.float32)
    wfull = W32 @ ft0 + b32[:, None]               # [H, N]
    ftr0 = np.ascontiguousarray(ft0, dtype=np.float16)
    ftr1 = np.ascontiguousarray(np.roll(ft0, -HALF, axis=1), dtype=np.float16)
    bT = bias0.T                                    # [j, i]
    W16 = W32.astype(np.float16)
    ins = []
    bt_cache = {}
    for core in range(NCORES):
        h = core % H
        off = core // H
        key = (h, off)
        if key not in bt_cache:
            btw = bT + 0.2 * wfull[h][None, :]
            if off:
                btw = np.roll(btw, (-HALF, -HALF), (0, 1))
            bt_cache[key] = btw.astype(np.float16)
        ins.append({
            "ftr": ftr1 if off else ftr0,
            "biasT": bt_cache[key],
            "wh": np.ascontiguousarray(W16[h]).reshape(1, -1),
            "bh": b32[h].reshape(1, 1),
        })
    return ins


def assemble_output_v3(results, n=N):
    """Unpack compact diagonal blocks, un-roll, mirror-fill (t symmetric)."""
    nb = NB
    full = np.zeros((1, n, H * n), np.float32)
    for h in range(H):
        tfull = np.zeros((n, n), np.float32)
        for off_i in range(2):
            o = results[h + H * off_i]["out"].astype(np.float32)
            o = o.reshape(HALF // 128, 128, DMAX + 1, 128)
            roff = off_i * NH
            for ib in range(NH):
                rb = (ib + roff) % nb
                for d in range(DMAX + 1):
                    kb = (ib + d + roff) % nb
                    tfull[rb * 128:(rb + 1) * 128,
                          kb * 128:(kb + 1) * 128] = o[ib, :, d]
        # mirror: cyclic distance >= DMAX+1 blocks come from the transpose
        for p in range(nb):
            for q in range(nb):
                if (q - p) % nb > DMAX:
                    tfull[p * 128:(p + 1) * 128, q * 128:(q + 1) * 128] = \
                        tfull[q * 128:(q + 1) * 128, p * 128:(p + 1) * 128].T
        full[0, :, h * n:(h + 1) * n] = tfull
    return full


_nc_cache = {}


def kernel(ft_mat, bias_mat, W, b):
    key = ("v3", N)
    if key not in _nc_cache:
        nc = build_kernel_v3()
        nc.finalize()
        _nc_cache[key] = nc
    nc = _nc_cache[key]
    ins = make_core_inputs_v3(ft_mat, bias_mat, W, b)
    res = run_bass_kernel_spmd(nc, ins, list(range(NCORES)))
    return assemble_output_v3(res.results)
